# revision 4
# baseline (speedup 1.0000x reference)
"""Trainium2 Bass kernel for nn_CustomFullyConnectedLayer (topk_masking).

Math: reference builds W[r, c] = a[(r-c)%N] * V[(r-c)%N, c]  (P=D=N=4096,
the scatter-add has no collisions), then y = x @ W.T.

So  y[b, r] = sum_c x[b, c] * a[(r-c)%N] * V[(r-c)%N, c].

Sharding (tensor parallel over out_features): core k owns output columns
r in [512k, 512k+512). Define S_k[c, t] = U[(512k+t-c)%N, c] with
U = a[:,None]*V — a wrapped-diagonal band holding exactly 1/8 of V.
Then y[:, 512k:512k+512] = x @ S_k, a plain dense matmul per core.

Host side: Dykstra soft-top-k mask (50 iters over a 4096-vector, trivial),
band gather via a strided view (layout only), fold the (uniform) mask
scalar into x. Device side: tiled 512x4096 @ 4096x512 matmul in fp16
(10-bit mantissa) with fp32 PSUM accumulation, running at the PE's full
216 ns / [128x128x512] instruction cadence.

Schedule notes (from NTFF traces):
- k-tile DMAs alternate between the two HW DGE queues (sync + scalar
  engines) so input never gates the PE stream and tile0 lands ~1.4 us
  after queue spin-up.
- 6 warm-up matmuls bridge the preamble->tile0 window and keep the PE
  busy through the HAM clock ramp (cold PE runs at half clock for the
  first ~3.4 us of cumulative busy time).
- Output is written m-interleaved ([128, 4*512] fp16, one partition row
  per batch-mod-128) so each of the two tail DMAs moves 2 KB/partition
  contiguous runs; PSUM->SBUF copies alternate vector/scalar engines and
  each column pair is flushed as soon as its two copies complete.
"""

from contextlib import ExitStack

import numpy as np

import concourse.bacc as bacc
import concourse.bass as bass
import concourse.mybir as mybir
import concourse.tile as tile
from concourse.bass_utils import run_bass_kernel_spmd

N = 4096          # in_features == out_features == P == D
B = 512           # batch
NCORES = 8
TBAND = N // NCORES  # 512 output columns per core
KT = 128          # contraction tile (partition dim)
NKT = N // KT     # 32
MT = 128          # batch tile (psum partition dim)
NMT = B // MT     # 4
NWARM = 26

TOPK = 3687       # ceil((1-0.1)*4096*4096/4096)
ALPHA_LR = 0.01
NUM_ITER = 50

_NC_CACHE = {}


def _dykstra_mask(alpha: np.ndarray) -> np.ndarray:
    """Faithful float32 replica of reference.sparse_soft_topk_mask_dykstra."""
    s = (alpha.astype(np.float32) / np.float32(ALPHA_LR)).astype(np.float32)
    n = np.float32(s.shape[0])
    k = np.float32(TOPK)
    x = s.copy()
    p = np.zeros_like(s)
    q = np.zeros_like(s)
    for _ in range(NUM_ITER):
        t = x + p
        y = t + (k - np.sum(t, dtype=np.float32)) / n
        p = t - y
        u = y + q
        xn = np.clip(u, np.float32(0.0), np.float32(1.0))
        q = u - xn
        x = xn
    return x


def _build_nc():
    f32 = mybir.dt.float32
    f16 = mybir.dt.float16

    nc = bacc.Bacc("TRN2", target_bir_lowering=False)
    # xs = [x.T | S_k] concatenated on host: one DMA per k-tile keeps every
    # DMA at 2 KB/partition rows (the efficient descriptor shape).
    xs = nc.dram_tensor("xs", [N, B + TBAND], f16, kind="ExternalInput")
    # m-interleaved output: y_dev[p, m*TBAND + c] = y[m*MT + p, c]
    y = nc.dram_tensor("y", [MT, NMT * TBAND], f16, kind="ExternalOutput")

    with tile.TileContext(nc) as tc, ExitStack() as ctx:
        xpool = ctx.enter_context(tc.tile_pool(name="xp", bufs=10))
        wpool = ctx.enter_context(tc.tile_pool(name="wp", bufs=1))
        opool = ctx.enter_context(tc.tile_pool(name="op", bufs=1))
        pspool = ctx.enter_context(tc.tile_pool(name="ps", bufs=1, space="PSUM"))

        ps = [pspool.tile([MT, TBAND], f32, tag=f"ps{m}", name=f"ps{m}") for m in range(NMT)]

        # PE warm-up bridging preamble -> tile0 arrival and the HAM clock
        # ramp. The tile is [128,128] so its memset (~200 ns on gpsimd) ends
        # before the Tensor engine's preamble does — a [128,512] memset was
        # the actual gate on the first matmul. Short 128-column warm-ups
        # (~107 ns each at ramp clock) keep PE-busy granular, so the bridge
        # overshoots tile0's semaphore by at most one warm-up.
        wu = wpool.tile([KT, KT], f16, tag="wu", name="wu")
        nc.gpsimd.memset(wu[:], 0.0)
        psw = pspool.tile([MT, KT], f32, tag="psw", name="psw")
        for w in range(NWARM):
            nc.tensor.matmul(psw[:], lhsT=wu[:], rhs=wu[:],
                             start=True, stop=True)

        for k in range(NKT):
            xst = xpool.tile([KT, B + TBAND], f16, tag="x", name=f"xst{k}")
            eng = nc.sync if k % 2 == 0 else nc.scalar
            eng.dma_start(out=xst[:], in_=xs[k * KT:(k + 1) * KT, :])
            for m in range(NMT):
                nc.tensor.matmul(
                    ps[m][:],
                    lhsT=xst[:, m * MT:(m + 1) * MT],
                    rhs=xst[:, B:B + TBAND],
                    start=(k == 0),
                    stop=(k == NKT - 1),
                )

        otp = opool.tile([MT, NMT * TBAND], f16, tag="o", name="otp")
        for m in range(NMT):
            dst = otp[:, m * TBAND:(m + 1) * TBAND]
            if m % 2 == 0:
                nc.vector.tensor_copy(dst, ps[m][:])
            else:
                nc.scalar.copy(dst, ps[m][:])
                # flush each completed column pair immediately, alternating
                # the two HW DGE queues
                deng = nc.sync if m == 1 else nc.scalar
                deng.dma_start(out=y[:, (m - 1) * TBAND:(m + 1) * TBAND],
                               in_=otp[:, (m - 1) * TBAND:(m + 1) * TBAND])

    nc.compile()
    return nc


def _get_nc():
    if "nc" not in _NC_CACHE:
        _NC_CACHE["nc"] = _build_nc()
    return _NC_CACHE["nc"]


def _prepare_in_maps(x, V, alpha):
    a = _dykstra_mask(np.asarray(alpha, dtype=np.float32))
    x = np.asarray(x, dtype=np.float32)
    V = np.asarray(V, dtype=np.float32)

    if np.all(a == a[0]):
        # uniform mask (the alpha=const case): fold the scalar into x
        xs = x * np.float32(a[0])
        U = V
    else:
        xs = x
        U = (a[:, None] * V).astype(np.float32)

    xT = np.ascontiguousarray(xs.T)  # [N, B]

    # doubled-rows copy so every wrapped-diagonal band is a plain strided view
    Vd = np.ascontiguousarray(np.concatenate([U, U], axis=0))  # [2N, N]
    flat = Vd.reshape(-1)

    in_maps = []
    for k in range(NCORES):
        base = (N + TBAND * k) * N
        # S_k[c, t] = Vd[N + 512k + t - c, c] = flat[base + c*(1-N) + t*N]
        Sk = np.lib.stride_tricks.as_strided(
            flat[base:], shape=(N, TBAND), strides=((1 - N) * 4, N * 4)
        )
        xs_k = np.concatenate([xT, Sk], axis=1).astype(np.float16)  # [N, B+TBAND]
        in_maps.append({"xs": np.ascontiguousarray(xs_k)})
    return in_maps


def _gather(results):
    cols = []
    for k in range(NCORES):
        yk = results[k]["y"].astype(np.float32)  # [MT, NMT*TBAND] fp16
        cols.append(yk.reshape(MT, NMT, TBAND).transpose(1, 0, 2).reshape(B, TBAND))
    return np.ascontiguousarray(np.concatenate(cols, axis=1))


def _run(in_maps, trace=False, **kw):
    nc = _get_nc()
    return run_bass_kernel_spmd(nc, in_maps, list(range(NCORES)), trace=trace, **kw)


def kernel(x, V, alpha):
    in_maps = _prepare_in_maps(x, V, alpha)
    res = _run(in_maps).results
    return _gather(res)
